# revision 1
# baseline (speedup 1.0000x reference)
"""Fused LayerNorm + multi-head attention + output projection on 8 TRN2 cores.

Sharding: core c handles batch b = c//4 and head group g = c%4 (4 of 16 heads).
Each core computes LN(x[b]) (replicated within the batch's 4 cores), the qkv
projection for its heads, attention, and a partial output projection (w_out
rows for its heads). The host sums the 4 partials per batch.

On-chip layout is fully transposed ([feature, token]); the host pre-transposes
x, folds gamma / softmax scale / beta into the weights, and packs everything in
SBUF-ready layouts, so the kernel needs zero on-chip transposes:

  xn^T   [D, T]   = LayerNorm(x)^T        (stats via ones-matmul broadcast)
  q^T/k^T [dh, T] = W_q/k^T-slices @ xn^T (feature-major)
  v      [T, dh]  = xn^T-tiles.T @ W_v    (token-major, swapped operands)
  E^T    [k, q]   = exp(K Q^T)            (no max subtraction: scores ~N(0,1))
  av^T   [dh, q]  = V-block @ E^T         (denominator row rides along free)
  out^T  [D, T]   = w_out-slice^T @ (av^T/den)
"""

import numpy as np

HEADS = 16
DIM_HEAD = 64
SCALE = DIM_HEAD**-0.5
EPS = 1e-5
B, S, D = 2, 2048, 1024
T = S
NCORES = 8
NH = 4  # heads per core
F = 3 * NH * DIM_HEAD  # 768 features per core: [q(256) | k(256) | v(256)]
DC = D // 128  # 8 contraction chunks
KC = T // 128  # 16 key chunks
QB = 4  # q blocks
QW = T // QB  # 512 q block width

_cache = {}


def _build():
    import concourse.bacc as bacc
    import concourse.mybir as mybir
    import concourse.tile as tile

    fp32 = mybir.dt.float32
    bf16 = mybir.dt.bfloat16
    AF = mybir.ActivationFunctionType
    ALU = mybir.AluOpType

    nc = bacc.Bacc("TRN2", target_bir_lowering=False, debug=False,
                   num_devices=NCORES)
    xt_d = nc.declare_dram_parameter("xt", [128, DC * T], bf16, isOutput=False)
    wqkv_d = nc.declare_dram_parameter("wqkv", [128, DC * F], bf16, isOutput=False)
    wout_d = nc.declare_dram_parameter("wout", [128, 4 * D], bf16, isOutput=False)
    bqkv_d = nc.declare_dram_parameter("bqkv", [1, F], bf16, isOutput=False)
    bqkc_d = nc.declare_dram_parameter("bqkc", [128, 4], fp32, isOutput=False)
    bout_d = nc.declare_dram_parameter("bout", [128, 8], fp32, isOutput=False)
    out_d = nc.declare_dram_parameter("out", [D, T], fp32, isOutput=True)
    dbg = {}
    if _cache.get("debug"):
        dbg["xn"] = nc.declare_dram_parameter("dbg_xn", [128, DC * T], bf16, isOutput=True)
        dbg["qk"] = nc.declare_dram_parameter("dbg_qk", [128, 4 * T], bf16, isOutput=True)
        dbg["vsb"] = nc.declare_dram_parameter("dbg_vsb", [128, KC * NH * 128], bf16, isOutput=True)
        dbg["aot"] = nc.declare_dram_parameter("dbg_aot", [128, 4 * T], bf16, isOutput=True)

    with tile.TileContext(nc) as tc:
        with (
            tc.tile_pool(name="const", bufs=1) as constp,
            tc.tile_pool(name="big", bufs=1) as bigp,
            tc.tile_pool(name="work", bufs=2) as workp,
            tc.tile_pool(name="psum", bufs=1, space="PSUM") as psump,
        ):
            # ---- persistent SBUF ----
            ones128 = constp.tile([128, 128], bf16, tag="ones128")
            nc.gpsimd.memset(ones128[:], 1.0)
            onesrow = constp.tile([1, QW], bf16, tag="onesrow")
            nc.gpsimd.memset(onesrow[:], 1.0)
            wqkv = constp.tile([128, DC * F], bf16, tag="wqkv")
            nc.sync.dma_start(wqkv[:], wqkv_d[:])
            wout = constp.tile([128, 4 * D], bf16, tag="wout")
            nc.sync.dma_start(wout[:], wout_d[:])
            bqkv = constp.tile([1, F], bf16, tag="bqkv")
            nc.sync.dma_start(bqkv[:], bqkv_d[:])
            bqkc = constp.tile([128, 4], fp32, tag="bqkc")
            nc.sync.dma_start(bqkc[:], bqkc_d[:])
            bout = constp.tile([128, 8], fp32, tag="bout")
            nc.sync.dma_start(bout[:], bout_d[:])

            xn = bigp.tile([128, DC * T], bf16, tag="xn")  # normalized x^T
            mean_b = bigp.tile([128, T], bf16, tag="mean_b")
            rstd_b = bigp.tile([128, T], bf16, tag="rstd_b")
            # q^T / k^T feature-major: m=0,1 -> q heads (0,1),(2,3); m=2,3 -> k
            qk = bigp.tile([128, 4 * T], bf16, tag="qk")
            # v blocks, 128 wide per (k-chunk, head), all heads HI-style:
            #   [ones(1) | zeros(63) | V(64)] -> den row 0, attn rows 64-127
            vsb = bigp.tile([128, KC * NH * 128], bf16, tag="vsb")
            nc.gpsimd.memset(vsb[:], 0.0)
            vsb_r = vsb[:].rearrange("p (c h o) -> p c h o", h=NH, o=128)
            nc.gpsimd.memset(vsb_r[:, :, :, 0:1], 1.0)
            # attention output^T: 4 chunks, head hh at rows 64-127 of chunk hh,
            # rows 0-63 stay zero (wout rows are zero-padded to match)
            aot = bigp.tile([128, 4 * T], bf16, tag="aot")
            nc.gpsimd.memset(aot[0:64, :], 0.0)

            # psum slots: 4 tags x [128, 1024] (2 banks each) = 8 banks
            ps_n = [0]

            def ps(tag):
                ps_n[0] += 1
                return psump.tile([128, 1024], fp32, tag=tag,
                                  name=f"ps_{tag}_{ps_n[0]}")

            ab = [0]

            def ps_ab():
                ab[0] += 1
                return ps(["psA", "psB"][ab[0] % 2])

            # ================= Phase 1: LayerNorm =================
            with (tc.tile_pool(name="ln", bufs=1) as lnp,
                  tc.tile_pool(name="lnw", bufs=2) as lnwp):
                xt = lnp.tile([128, DC * T], bf16, tag="xt")
                for c in range(DC):
                    csl = slice(c * T, (c + 1) * T)
                    nc.sync.dma_start(xt[:, csl], xt_d[:, csl])
                x2 = xn  # scratch: stats are done before xn is written
                for c in range(DC):
                    csl = slice(c * T, (c + 1) * T)
                    nc.scalar.activation(x2[:, csl], xt[:, csl], AF.Square)
                for tb in range(4):
                    slot = ps_ab()
                    s_ps, q_ps = slot[:, 0:512], slot[:, 512:1024]
                    for c in range(DC):
                        sl = slice(c * T + tb * 512, c * T + (tb + 1) * 512)
                        nc.tensor.matmul(s_ps, ones128[:], xt[:, sl],
                                         start=(c == 0), stop=(c == DC - 1))
                    for c in range(DC):
                        sl = slice(c * T + tb * 512, c * T + (tb + 1) * 512)
                        nc.tensor.matmul(q_ps, ones128[:], x2[:, sl],
                                         start=(c == 0), stop=(c == DC - 1))
                    tsl = slice(tb * 512, (tb + 1) * 512)
                    nc.vector.tensor_scalar(out=mean_b[:, tsl], in0=s_ps,
                                            scalar1=1.0 / D, scalar2=None,
                                            op0=ALU.mult)
                    t1 = lnwp.tile([128, 512], fp32, tag="lnt1")
                    nc.vector.tensor_scalar(out=t1[:], in0=q_ps,
                                            scalar1=1.0 / D, scalar2=EPS,
                                            op0=ALU.mult, op1=ALU.add)
                    m2 = lnwp.tile([128, 512], fp32, tag="lnm2")
                    nc.vector.tensor_tensor(out=m2[:], in0=mean_b[:, tsl],
                                            in1=mean_b[:, tsl], op=ALU.mult)
                    var = lnwp.tile([128, 512], fp32, tag="lnvar")
                    nc.vector.tensor_tensor(out=var[:], in0=t1[:], in1=m2[:],
                                            op=ALU.subtract)
                    lnv = lnwp.tile([128, 512], fp32, tag="lnlnv")
                    nc.scalar.activation(lnv[:], var[:], AF.Ln)
                    nc.scalar.activation(rstd_b[:, tsl], lnv[:], AF.Exp,
                                         scale=-0.5)
                # normalize chunk-by-chunk so the qkv matmuls (which
                # consume whole chunks) can start as early as possible
                for c in range(DC):
                    csl = slice(c * T, (c + 1) * T)
                    xc = lnwp.tile([128, T], bf16, tag="lnxc")
                    nc.vector.tensor_tensor(out=xc[:], in0=xt[:, csl],
                                            in1=mean_b[:], op=ALU.subtract)
                    nc.vector.tensor_tensor(out=xn[:, csl], in0=xc[:],
                                            in1=rstd_b[:], op=ALU.mult)

            # ============ Phase 2: QKV, attention, out projection ============
            with tc.tile_pool(name="attn", bufs=1) as attnp:
                def qk_proj(m):
                    for tbp in range(2):  # tb pairs: (0,1), (2,3)
                        slot = ps_ab()
                        for half in range(2):
                            tb = tbp * 2 + half
                            o = slot[:, half * 512:(half + 1) * 512]
                            for c in range(DC):
                                nc.tensor.matmul(
                                    o,
                                    wqkv[:, c * F + m * 128:c * F + (m + 1) * 128],
                                    xn[:, c * T + tb * 512:c * T + (tb + 1) * 512],
                                    start=(c == 0), stop=(c == DC - 1))
                        nc.vector.tensor_scalar(
                            out=qk[:, m * T + tbp * 1024:m * T + (tbp + 1) * 1024],
                            in0=slot[:], scalar1=bqkc[:, m:m + 1], scalar2=None,
                            op0=ALU.add)

                def qk_proj_late(m, tbp):
                    if True:
                        slot = ps("psD")
                        for half in range(2):
                            tb = tbp * 2 + half
                            o = slot[:, half * 512:(half + 1) * 512]
                            for c in range(DC):
                                nc.tensor.matmul(
                                    o,
                                    wqkv[:, c * F + m * 128:c * F + (m + 1) * 128],
                                    xn[:, c * T + tb * 512:c * T + (tb + 1) * 512],
                                    start=(c == 0), stop=(c == DC - 1))
                        nc.vector.tensor_scalar(
                            out=qk[:, m * T + tbp * 1024:m * T + (tbp + 1) * 1024],
                            in0=slot[:], scalar1=bqkc[:, m:m + 1], scalar2=None,
                            op0=ALU.add)

                def v_proj_grp(tq):
                    if True:
                        slot = ps("psD")
                        for half in range(4):
                            tt = tq * 4 + half
                            o = slot[:, half * 256:(half + 1) * 256]
                            nc.tensor.matmul(o, onesrow[0:1, 0:128],
                                             bqkv[0:1, 512:768],
                                             start=True, stop=False)
                            for c in range(DC):
                                nc.tensor.matmul(
                                    o,
                                    xn[:, c * T + tt * 128:c * T + (tt + 1) * 128],
                                    wqkv[:, c * F + 512:c * F + 768],
                                    start=False, stop=(c == DC - 1))
                        src = slot[:].rearrange("p (q h d) -> p q h d", q=4, h=NH)
                        nc.vector.tensor_copy(
                            out=vsb_r[:, tq * 4:(tq + 1) * 4, :, 64:128],
                            in_=src[:])

                qk_proj(0)
                qk_proj(2)

                # --- attention + pipelined normalize + output projection ---
                eblk0 = attnp.tile([128, KC * 1024], bf16, tag="eblk0")
                eblk1 = attnp.tile([128, KC * 1024], bf16, tag="eblk1")
                LAG = 2

                def normalize(blk):
                    qb, pair, av = blk
                    # den_h0 | den_h1 live contiguously at row 0
                    rc = workp.tile([128, 1024], fp32, tag="recf")
                    nc.vector.reciprocal_approx_fast(
                        out=rc[0:1, :], in_=av[0:1, :])
                    rcb = workp.tile([128, 1024], bf16, tag="recb")
                    nc.vector.tensor_copy(out=rcb[0:1, :], in_=rc[0:1, :])
                    rbc = workp.tile([128, 1024], bf16, tag="rbcs")
                    nc.gpsimd.partition_broadcast(rbc[:], rcb[0:1, :])
                    un = workp.tile([128, 1024], bf16, tag="avun")
                    nc.vector.tensor_copy(out=un[64:128, :], in_=av[64:128, :])
                    # one TT, strided out: head hh -> chunk hh rows 64-127
                    dst = aot[64:128, :].rearrange("p (hh t) -> p hh t", hh=4)
                    nc.vector.tensor_tensor(
                        out=dst[:, 2 * pair:2 * pair + 2,
                                qb * QW:(qb + 1) * QW],
                        in0=un[64:128, :].rearrange("p (hh t) -> p hh t", hh=2),
                        in1=rbc[64:128, :].rearrange("p (hh t) -> p hh t", hh=2),
                        op=ALU.mult)

                def outproj(qb):
                    for mp in range(4):
                        outproj_grp(qb, mp, tag=["psA", "psB"][mp % 2])

                def outproj_grp(qb, mp, tag="psD"):
                    qsl = slice(qb * QW, (qb + 1) * QW)
                    if True:
                        slot = ps(tag)
                        for half in range(2):
                            m = 2 * mp + half
                            o = slot[:, half * 512:(half + 1) * 512]
                            for c2 in range(4):
                                nc.tensor.matmul(
                                    o,
                                    wout[:, c2 * D + m * 128:c2 * D + (m + 1) * 128],
                                    aot[:, c2 * T + qb * QW:c2 * T + (qb + 1) * QW],
                                    start=(c2 == 0), stop=(c2 == 3))
                        ob = workp.tile([128, 1024], fp32, tag="ob")
                        for half in range(2):
                            m = 2 * mp + half
                            nc.vector.tensor_scalar(
                                out=ob[:, half * 512:(half + 1) * 512],
                                in0=slot[:, half * 512:(half + 1) * 512],
                                scalar1=bout[:, m:m + 1], scalar2=None,
                                op0=ALU.add)
                        for half in range(2):
                            m = 2 * mp + half
                            nc.sync.dma_start(
                                out_d[m * 128:(m + 1) * 128, qsl],
                                ob[:, half * 512:(half + 1) * 512])

                pending = None
                startup = [lambda tq=tq: v_proj_grp(tq) for tq in range(4)]
                startup += [lambda m=m, tbp=tbp: qk_proj_late(m, tbp)
                            for m in (1, 3) for tbp in range(2)]
                fillers = startup
                block_order = [(0, 0), (1, 0), (0, 1), (1, 1),
                               (2, 0), (2, 1), (3, 0), (3, 1)]
                prev = None
                for bi, (qb, pair) in enumerate(block_order):
                    qsl = slice(qb * QW, (qb + 1) * QW)
                    eblk = (eblk0, eblk1)[bi % 2]
                    qm = qk[:, (0 + pair) * T:(1 + pair) * T]
                    km = qk[:, (2 + pair) * T:(3 + pair) * T]
                    if prev is not None:
                        pqb, ppair, peblk = prev
                        pav = ps("psC")

                        def av_mms(c, av=pav, pair=ppair, eblk=peblk):
                            for h in range(2):
                                hh = pair * 2 + h
                                nc.tensor.matmul(
                                    av[:, h * 512:(h + 1) * 512],
                                    vsb[:, (c * NH + hh) * 128:(c * NH + hh + 1) * 128],
                                    eblk[:, c * 1024 + h * 512:c * 1024 + (h + 1) * 512],
                                    start=(c == 0), stop=(c == KC - 1))
                    for c in range(KC):
                        if prev is not None:
                            av_mms(c)
                        eps_ = ps_ab()
                        ksl = slice(c * 128, (c + 1) * 128)
                        nc.tensor.matmul(eps_[:, 0:512], km[0:64, ksl],
                                         qm[0:64, qsl],
                                         tile_position=(0, 0))
                        nc.tensor.matmul(eps_[:, 512:1024], km[64:128, ksl],
                                         qm[64:128, qsl],
                                         tile_position=(64, 0))
                        nc.scalar.activation(
                            eblk[:, c * 1024:(c + 1) * 1024], eps_[:],
                            AF.Exp)
                        fcad = 4 if bi < 2 else 5
                        if c % fcad == fcad - 1 and fillers:
                            fillers.pop(0)()
                    if prev is not None:
                        blk = (pqb, ppair, pav)
                        fillers.append(lambda blk=blk: normalize(blk))
                        if ppair == 1:
                            fillers.extend(
                                [lambda q=pqb, mp=mp: outproj_grp(q, mp)
                                 for mp in range(4)])
                    prev = (qb, pair, eblk)
                # tail: AV + normalize of the last block, remaining fillers
                pqb, ppair, peblk = prev
                pav = ps("psC")
                for c in range(KC):
                    for h in range(2):
                        hh = ppair * 2 + h
                        nc.tensor.matmul(
                            pav[:, h * 512:(h + 1) * 512],
                            vsb[:, (c * NH + hh) * 128:(c * NH + hh + 1) * 128],
                            peblk[:, c * 1024 + h * 512:c * 1024 + (h + 1) * 512],
                            start=(c == 0), stop=(c == KC - 1))
                    if c % 3 == 2 and fillers:
                        fillers.pop(0)()
                while fillers:
                    fillers.pop(0)()
                normalize((pqb, ppair, pav))
                outproj(pqb)
                if dbg:
                    nc.sync.dma_start(dbg["xn"][:], xn[:])
                    nc.sync.dma_start(dbg["qk"][:], qk[:])
                    nc.sync.dma_start(dbg["vsb"][:], vsb[:])
                    nc.sync.dma_start(dbg["aot"][:], aot[:])

    nc.compile()
    return nc


def _prep_inputs(x, gamma, beta, w_qkv, w_out, b_out):
    import ml_dtypes

    bf16 = ml_dtypes.bfloat16
    wg = (w_qkv * gamma[:, None]).astype(np.float32)  # fold gamma
    bias_full = (beta @ w_qkv).astype(np.float32)  # fold beta
    in_maps = []
    for core in range(NCORES):
        b, g = divmod(core, 4)
        cs = slice(g * 256, (g + 1) * 256)
        qc = wg[:, 0 * D:1 * D][:, cs] * SCALE
        kc = wg[:, 1 * D:2 * D][:, cs]
        vc = wg[:, 2 * D:3 * D][:, cs]
        w_core = np.concatenate([qc, kc, vc], axis=1)  # [1024, 768]
        bq = bias_full[0 * D:1 * D][cs] * SCALE
        bk = bias_full[1 * D:2 * D][cs]
        bv = bias_full[2 * D:3 * D][cs]
        b_core = np.concatenate([bq, bk, bv])[None, :]  # [1, 768]
        xt = np.ascontiguousarray(x[b].T)  # [1024, 2048]
        xt_sb = xt.reshape(DC, 128, T).transpose(1, 0, 2).reshape(128, DC * T)
        wqkv_sb = w_core.reshape(DC, 128, F).transpose(1, 0, 2).reshape(128, DC * F)
        wout_core = w_out[g * 256:(g + 1) * 256, :]  # [256, 1024]
        wout_sb = np.zeros((128, 4 * D), np.float32)
        for c2 in range(4):
            wout_sb[64:128, c2 * D:(c2 + 1) * D] = wout_core[c2 * 64:(c2 + 1) * 64]
        in_maps.append({
            "xt": np.ascontiguousarray(xt_sb).astype(bf16),
            "wqkv": np.ascontiguousarray(wqkv_sb).astype(bf16),
            "wout": np.ascontiguousarray(wout_sb).astype(bf16),
            "bqkv": np.ascontiguousarray(b_core).astype(bf16),
            "bqkc": np.ascontiguousarray(b_core[0, :512].reshape(4, 128).T).astype(np.float32),
            "bout": np.ascontiguousarray(b_out.reshape(8, 128).T).astype(np.float32),
        })
    return in_maps


def kernel(x, gamma, beta, w_qkv, w_out, b_out, _want_trace=False):
    from concourse.bass_utils import run_bass_kernel_spmd

    x = np.asarray(x, dtype=np.float32)
    gamma = np.asarray(gamma, dtype=np.float32)
    beta = np.asarray(beta, dtype=np.float32)
    w_qkv = np.asarray(w_qkv, dtype=np.float32)
    w_out = np.asarray(w_out, dtype=np.float32)
    b_out = np.asarray(b_out, dtype=np.float32)

    if "nc" not in _cache:
        _cache["nc"] = _build()
    nc = _cache["nc"]
    in_maps = _prep_inputs(x, gamma, beta, w_qkv, w_out, b_out)
    res = run_bass_kernel_spmd(nc, in_maps, core_ids=list(range(NCORES)),
                               trace=_want_trace)
    _cache["last_result"] = res
    out = np.empty((B, S, D), dtype=np.float32)
    for b in range(B):
        acc = np.zeros((D, T), dtype=np.float32)
        for g in range(4):
            acc += res.results[b * 4 + g]["out"]
        out[b] = acc.T
    return out

